# revision 30
# baseline (speedup 1.0000x reference)
"""GRU encoder + autoregressive decoder (seq2seq RNN) on 8 TRN2 cores.

Strategy: data-parallel over batch (512 -> 8 x 64), weights replicated.
Per core, the sequential recurrence runs locally:
  - h state kept BOTH in batch-major SBUF layout [64, 512] (for elementwise)
    and transposed hT [128, 4, 64] (stationary operand for matmuls).
  - Matmuls: out[batch, gate_cols] = hT.T @ WhhT with moving weights stored
    as float32r (FP22) -> 1 cycle/row at moving-free >= 256.
  - Biases folded into the matmuls via an appended ones-row on the
    ih-stationary (x / y) and a [1,512] bhh_n moving row.
  - sigmoid(x) computed as 0.5*tanh(0.5x)+0.5 so only the Tanh ACT table is
    used (avoids activation-table swap penalties).
  - h' = n*u + 0.5*(tz*h + h), u = 0.5 - 0.5*tz, tz = tanh(0.5*z_pre).
  - h' -> hT via 4 PE transposes per step.
"""

import numpy as np

N_CORES = 8
B = 64           # batch per core
T = 128          # encoder steps
I = 64           # input size
H = 512          # hidden size
O = 64           # output size
L = 300          # predict length
NK = H // 128    # K tiles over hidden dim
G = 512          # gate region width (cols per gate, = H)

_CACHE = {}
LAST_RESULTS = None


def _build_nc():
    from contextlib import ExitStack

    import concourse.bass as bass
    import concourse.tile as tile
    from concourse import bacc, mybir
    from concourse.masks import make_identity

    f32 = mybir.dt.float32
    f32r = mybir.dt.float32r
    AF = mybir.ActivationFunctionType
    ALU = mybir.AluOpType

    nc = bacc.Bacc(trn_type="TRN2")

    x_d = nc.dram_tensor("x", [T, I + 1, B], f32r, kind="ExternalInput")
    whh_d = nc.dram_tensor("whh", [H, 3 * H], f32r, kind="ExternalInput")
    wih_d = nc.dram_tensor("wih", [I + 1, 3 * H], f32r, kind="ExternalInput")
    bhhn_d = nc.dram_tensor("bhhn", [1, H], f32r, kind="ExternalInput")
    wo_d = nc.dram_tensor("wo", [H, O], f32r, kind="ExternalInput")
    bo_d = nc.dram_tensor("bo", [1, O], f32, kind="ExternalInput")
    out_d = nc.dram_tensor("out", [B, L * O], f32, kind="ExternalOutput")

    with tile.TileContext(nc) as tc, ExitStack() as ctx:
        singles = ctx.enter_context(tc.tile_pool(name="singles", bufs=1))
        xpool = ctx.enter_context(tc.tile_pool(name="xpool", bufs=3))
        gpsum = ctx.enter_context(tc.tile_pool(name="gpsum", bufs=1, space="PSUM"))
        tpsum = ctx.enter_context(tc.tile_pool(name="tpsum", bufs=2, space="PSUM"))
        ypsum = ctx.enter_context(tc.tile_pool(name="ypsum", bufs=1, space="PSUM"))

        dma = nc.default_dma_engine

        # --- weights / constants ---
        whh = singles.tile([128, NK, 3 * H], f32r)
        dma.dma_start(whh[:], whh_d[:].rearrange("(k p) j -> p k j", p=128))
        wih = singles.tile([I + 1, 3 * H], f32r)
        dma.dma_start(wih[:], wih_d[:])
        bhhn = singles.tile([1, H], f32r)
        dma.dma_start(bhhn[:], bhhn_d[:])
        wo = singles.tile([128, NK, O], f32r)
        dma.dma_start(wo[:], wo_d[:].rearrange("(k p) o -> p k o", p=128))
        bo_bc = singles.tile([B, O], f32)
        bo_ap = bo_d[:]
        dma.dma_start(
            bo_bc[:],
            bass.AP(tensor=bo_ap.tensor, offset=bo_ap.offset,
                    ap=[[0, B], list(bo_ap.ap[-1])]),
        )

        ones_f = singles.tile([1, B], f32)
        nc.vector.memset(ones_f[:], 1.0)
        ones_s = singles.tile([1, B], f32r)
        nc.scalar.activation(ones_s[:], ones_f[:], AF.Copy)
        half_b = singles.tile([B, 1], f32)
        nc.vector.memset(half_b[:], 0.5)
        zero_b = singles.tile([B, 1], f32)
        nc.vector.memset(zero_b[:], 0.0)
        ident = singles.tile([B, B], f32)
        make_identity(nc, ident[:])

        # --- state ---
        h_sb = singles.tile([B, H], f32)
        nc.vector.memset(h_sb[:], 0.0)
        hz = singles.tile([128, NK, B], f32)
        nc.vector.memset(hz[:], 0.0)
        hT = singles.tile([128, NK, B], f32r)
        nc.scalar.activation(hT[:], hz[:], AF.Copy)
        yz = singles.tile([O + 1, B], f32)
        nc.vector.memset(yz[:], 0.0)
        nc.vector.memset(yz[O:O + 1, :], 1.0)
        yaug = singles.tile([O + 1, B], f32r)
        nc.scalar.activation(yaug[:], yz[:], AF.Copy)
        out_buf = singles.tile([B, L * O], f32)

        # --- elementwise temporaries (persistent, reused each step) ---
        tz = singles.tile([B, H], f32)
        u = singles.tile([B, H], f32)
        w1 = singles.tile([B, H], f32)
        s = singles.tile([B, H], f32)
        sh = singles.tile([B, H], f32)
        hnh = singles.tile([B, H], f32)
        q = singles.tile([B, H], f32)
        tr = singles.tile([B, H], f32)
        m1 = singles.tile([B, H], f32)
        arg = singles.tile([B, H], f32)
        nn = singles.tile([B, H], f32)
        nu = singles.tile([B, H], f32)

        HG = G // 2  # column half within each gate region

        def gru_step(x_stat):
            """One GRU step: h (h_sb/hT) updated in place.
            x_stat: [I+1, B] f32r stationary (input with ones row).
            Column-halved: PE matmuls for half 1 overlap ACT/DVE/Pool
            elementwise for half 0. Gate order r,hn,inn,z inside each half
            starts the tanh chain while later gates still stream."""
            g = gpsum.tile([B, 4, G], f32)  # regions: 0=r_pre 1=z_pre 2=inn 3=hn
            tp = tpsum.tile([128, NK, B], f32)
            for half in range(2):
                c0 = half * HG
                c1 = c0 + HG
                sl = slice(c0, c1)
                # r gate (closes first so tr can start early)
                for k in range(NK):
                    nc.tensor.matmul(g[:, 0, c0:c1], hT[:, k, :],
                                     whh[:, k, c0:c1], start=(k == 0), stop=False)
                nc.tensor.matmul(g[:, 0, c0:c1], x_stat, wih[:, c0:c1],
                                 start=False, stop=True)
                # hn gate
                for k in range(NK):
                    nc.tensor.matmul(g[:, 3, c0:c1], hT[:, k, :],
                                     whh[:, k, 2 * G + c0:2 * G + c1],
                                     start=(k == 0), stop=False)
                nc.tensor.matmul(g[:, 3, c0:c1], ones_s[:], bhhn[:, c0:c1],
                                 start=False, stop=True)
                # inn
                nc.tensor.matmul(g[:, 2, c0:c1], x_stat,
                                 wih[:, 2 * G + c0:2 * G + c1],
                                 start=True, stop=True)
                # z gate last (its consumers are the chain tail)
                for k in range(NK):
                    nc.tensor.matmul(g[:, 1, c0:c1], hT[:, k, :],
                                     whh[:, k, G + c0:G + c1],
                                     start=(k == 0), stop=False)
                nc.tensor.matmul(g[:, 1, c0:c1], x_stat, wih[:, G + c0:G + c1],
                                 start=False, stop=True)

                # r-route: n = tanh(inn + 0.5*hn + tr*(0.5*hn))
                nc.scalar.activation(tr[:, sl], g[:, 0, c0:c1], AF.Tanh,
                                     bias=zero_b[:], scale=0.5)
                nc.scalar.activation(hnh[:, sl], g[:, 3, c0:c1], AF.Identity,
                                     bias=zero_b[:], scale=0.5)
                nc.vector.tensor_mul(m1[:, sl], tr[:, sl], hnh[:, sl])
                nc.vector.tensor_add(q[:, sl], g[:, 2, c0:c1], hnh[:, sl])
                nc.vector.tensor_add(arg[:, sl], m1[:, sl], q[:, sl])
                nc.scalar.activation(nn[:, sl], arg[:, sl], AF.Tanh,
                                     bias=zero_b[:])
                # z-route: z = 0.5*tanh(0.5*z_pre) + 0.5 ; u = 1-z
                nc.scalar.activation(tz[:, sl], g[:, 1, c0:c1], AF.Tanh,
                                     bias=zero_b[:], scale=0.5)
                nc.scalar.activation(u[:, sl], tz[:, sl], AF.Identity,
                                     bias=half_b[:], scale=-0.5)
                # z*h path on Pool (off critical chain)
                nc.gpsimd.tensor_mul(w1[:, sl], tz[:, sl], h_sb[:, sl])
                nc.gpsimd.tensor_add(s[:, sl], w1[:, sl], h_sb[:, sl])
                nc.scalar.activation(sh[:, sl], s[:, sl], AF.Identity,
                                     bias=zero_b[:], scale=0.5)
                nc.vector.tensor_mul(nu[:, sl], nn[:, sl], u[:, sl])
                # h' = n*u + 0.5*(w1 + h) -- in place
                nc.vector.tensor_add(h_sb[:, sl], nu[:, sl], sh[:, sl])

            # transposes after both halves so PE never stalls mid-step;
            # half-0 transposes overlap half-1 elementwise.
            for c in range(NK):
                nc.tensor.transpose(tp[:, c, :], h_sb[:, c * 128:(c + 1) * 128],
                                    ident[:])
            for c in range(NK):
                nc.scalar.activation(hT[:, c, :], tp[:, c, :], AF.Copy)

        # persistent PSUM scratch: cols 0:B = decoder y matmul out [B, O],
        # cols B:2B = y transpose out [O, B].
        ydual = ypsum.tile([B, 2 * B], f32)

        # ---------------- encoder ----------------
        for t in range(T):
            x_t = xpool.tile([I + 1, B], f32r)
            dma.dma_start(x_t[:], x_d[t])
            gru_step(x_t[:])

        # ---------------- decoder ----------------
        for t in range(L):
            yp = ydual[:, 0:B]
            for k in range(NK):
                nc.tensor.matmul(yp, hT[:, k, :], wo[:, k, :],
                                 start=(k == 0), stop=(k == NK - 1))
            ysl = out_buf[:, t * O:(t + 1) * O]
            nc.vector.tensor_add(ysl, yp, bo_bc[:])
            if t < L - 1:
                ytp = ydual[:, B:2 * B]
                nc.tensor.transpose(ytp, ysl, ident[:])
                nc.scalar.activation(yaug[0:O, :], ytp, AF.Copy)
                gru_step(yaug[:])

        dma.dma_start(out_d[:], out_buf[:])

    nc.finalize()
    return nc


def _prep_in_maps(input_, Wih, Whh, bih, bhh, Wo, bo):
    f32 = np.float32
    input_ = np.asarray(input_, f32)
    Wih = np.asarray(Wih, f32)
    Whh = np.asarray(Whh, f32)
    bih = np.asarray(bih, f32)
    bhh = np.asarray(bhh, f32)
    Wo = np.asarray(Wo, f32)
    bo = np.asarray(bo, f32)

    whhT = np.ascontiguousarray(Whh.T)                    # [H, 3H]
    wih_aug = np.empty((I + 1, 3 * H), f32)
    wih_aug[:I] = Wih.T                                   # [I, 3H]
    bias_row = bih.copy()
    bias_row[: 2 * H] += bhh[: 2 * H]                     # r,z get bih+bhh
    wih_aug[I] = bias_row                                 # n gets bih only
    bhhn = np.ascontiguousarray(bhh[2 * H:].reshape(1, H))
    woT = np.ascontiguousarray(Wo.T)                      # [H, O]
    bo_r = np.ascontiguousarray(bo.reshape(1, O))

    shared = {"whh": whhT, "wih": wih_aug, "bhhn": bhhn, "wo": woT, "bo": bo_r}
    in_maps = []
    for c in range(N_CORES):
        xc = input_[c * B:(c + 1) * B]                    # [B, T, I]
        xaug = np.empty((T, I + 1, B), f32)
        xaug[:, :I, :] = xc.transpose(1, 2, 0)
        xaug[:, I, :] = 1.0
        in_maps.append({"x": np.ascontiguousarray(xaug), **shared})
    return in_maps


def kernel(**inputs):
    global LAST_RESULTS
    from concourse.bass_utils import run_bass_kernel_spmd

    pl = int(np.asarray(inputs["predict_length"]))
    assert pl == L, f"kernel compiled for predict_length={L}, got {pl}"

    in_maps = _prep_in_maps(
        inputs["input"], inputs["Wih"], inputs["Whh"],
        inputs["bih"], inputs["bhh"], inputs["Wo"], inputs["bo"],
    )

    if "nc" not in _CACHE:
        _CACHE["nc"] = _build_nc()
    nc = _CACHE["nc"]

    res = run_bass_kernel_spmd(nc, in_maps, core_ids=list(range(N_CORES)))
    LAST_RESULTS = res

    out = np.empty((N_CORES * B, L, O), np.float32)
    for c in range(N_CORES):
        out[c * B:(c + 1) * B] = res.results[c]["out"].reshape(B, L, O)
    return out


# revision 33
# speedup vs baseline: 1.2135x; 1.2135x over previous
"""GRU encoder + autoregressive decoder (seq2seq RNN) on 8 TRN2 cores.

Strategy: data-parallel over batch (512 -> 8 x 64), weights replicated.
Per core, the sequential recurrence runs locally:
  - h state kept BOTH in batch-major SBUF layout [64, 512] (for elementwise)
    and transposed hT [128, 4, 64] (stationary operand for matmuls).
  - Matmuls: out[batch, gate_cols] = hT.T @ WhhT with moving weights stored
    as float32r (FP22) -> 1 cycle/row at moving-free >= 256.
  - Biases folded into the matmuls via an appended ones-row on the
    ih-stationary (x / y) and a [1,512] bhh_n moving row.
  - n-gate recurrent weights and bhh_n are pre-scaled by 0.5 host-side so
    g3 = 0.5*hn directly (no scaling op on device).
  - sigmoid(x) computed as 0.5*tanh(0.5x)+0.5 so only the Tanh ACT table is
    used. u = -0.5*tz + 0.5 in one DVE tensor_scalar op.
  - A running h2 = 0.5*h state makes the z-path sh = tz*h2 + h2 (two Pool
    ops), keeping ACT for the three tanh/copy ops only.
  - Decoder output y^T = Wo.T-chunks^T @ hT computed directly in [O,B]
    layout (no transpose); bias added in the ACT that converts to f32r.
"""

import numpy as np

N_CORES = 8
B = 64           # batch per core
T = 128          # encoder steps
I = 64           # input size
H = 512          # hidden size
O = 64           # output size
L = 300          # predict length
NK = H // 128    # K tiles over hidden dim
G = 512          # gate region width (cols per gate, = H)

_CACHE = {}
LAST_RESULTS = None


def _build_nc():
    from contextlib import ExitStack

    import concourse.bass as bass
    import concourse.tile as tile
    from concourse import bacc, mybir
    from concourse.masks import make_identity

    f32 = mybir.dt.float32
    f32r = mybir.dt.float32r
    AF = mybir.ActivationFunctionType
    ALU = mybir.AluOpType

    nc = bacc.Bacc(trn_type="TRN2")

    x_d = nc.dram_tensor("x", [T, I + 1, B], f32r, kind="ExternalInput")
    whh_d = nc.dram_tensor("whh", [H, 3 * H], f32r, kind="ExternalInput")
    wih_d = nc.dram_tensor("wih", [I + 1, 3 * H], f32r, kind="ExternalInput")
    bhhn_d = nc.dram_tensor("bhhn", [1, H], f32r, kind="ExternalInput")
    wo_d = nc.dram_tensor("wo", [H, O], f32r, kind="ExternalInput")
    bo_d = nc.dram_tensor("bo", [O, 1], f32, kind="ExternalInput")
    out_d = nc.dram_tensor("out", [O, L * B], f32, kind="ExternalOutput")

    with tile.TileContext(nc) as tc, ExitStack() as ctx:
        singles = ctx.enter_context(tc.tile_pool(name="singles", bufs=1))
        xpool = ctx.enter_context(tc.tile_pool(name="xpool", bufs=3))
        gpsum = ctx.enter_context(tc.tile_pool(name="gpsum", bufs=1, space="PSUM"))
        tpsum = ctx.enter_context(tc.tile_pool(name="tpsum", bufs=2, space="PSUM"))
        ypsum = ctx.enter_context(tc.tile_pool(name="ypsum", bufs=1, space="PSUM"))

        dma = nc.default_dma_engine

        # --- weights / constants ---
        whh = singles.tile([128, NK, 3 * H], f32r)
        dma.dma_start(whh[:], whh_d[:].rearrange("(k p) j -> p k j", p=128))
        wih = singles.tile([I + 1, 3 * H], f32r)
        dma.dma_start(wih[:], wih_d[:])
        bhhn = singles.tile([1, H], f32r)
        dma.dma_start(bhhn[:], bhhn_d[:])
        wo = singles.tile([128, NK, O], f32r)
        dma.dma_start(wo[:], wo_d[:].rearrange("(k p) o -> p k o", p=128))
        boT = singles.tile([O, 1], f32)
        dma.dma_start(boT[:], bo_d[:])

        ones_f = singles.tile([1, B], f32)
        nc.vector.memset(ones_f[:], 1.0)
        ones_s = singles.tile([1, B], f32r)
        nc.scalar.activation(ones_s[:], ones_f[:], AF.Copy)
        zero_b = singles.tile([B, 1], f32)
        nc.vector.memset(zero_b[:], 0.0)
        ident = singles.tile([B, B], f32)
        make_identity(nc, ident[:])

        # --- state ---
        h_sb = singles.tile([B, H], f32)
        nc.vector.memset(h_sb[:], 0.0)
        h2 = singles.tile([B, H], f32)       # running 0.5*h
        nc.vector.memset(h2[:], 0.0)
        hz = singles.tile([128, NK, B], f32)
        nc.vector.memset(hz[:], 0.0)
        hT = singles.tile([128, NK, B], f32r)
        nc.scalar.activation(hT[:], hz[:], AF.Copy)
        yz = singles.tile([O + 1, B], f32)
        nc.vector.memset(yz[:], 0.0)
        nc.vector.memset(yz[O:O + 1, :], 1.0)
        yaug = singles.tile([O + 1, B], f32r)
        nc.scalar.activation(yaug[:], yz[:], AF.Copy)
        outT = singles.tile([O, L * B], f32)

        # --- elementwise temporaries (persistent, reused each step) ---
        tz = singles.tile([B, H], f32)
        u = singles.tile([B, H], f32)
        w1 = singles.tile([B, H], f32)
        sh = singles.tile([B, H], f32)
        q = singles.tile([B, H], f32)
        tr = singles.tile([B, H], f32)
        m1 = singles.tile([B, H], f32)
        arg = singles.tile([B, H], f32)
        nn = singles.tile([B, H], f32)
        nu = singles.tile([B, H], f32)

        def gru_step(x_stat):
            """One GRU step: h (h_sb/hT/h2) updated in place.
            x_stat: [I+1, B] f32r stationary (input with ones row).
            g regions: 0=r_pre 1=z_pre 2=inn 3=0.5*hn (weights pre-scaled)."""
            g = gpsum.tile([B, 4, G], f32)
            tp = tpsum.tile([128, NK, B], f32)

            # x/bias parts first: no h dependency -> PE runs these while the
            # previous step's elementwise tail is still draining.
            nc.tensor.matmul(g[:, 1, :], x_stat, wih[:, G:2 * G],
                             start=True, stop=False)
            nc.tensor.matmul(g[:, 0, :], x_stat, wih[:, 0:G],
                             start=True, stop=False)
            nc.tensor.matmul(g[:, 2, :], x_stat, wih[:, 2 * G:3 * G],
                             start=True, stop=True)
            nc.tensor.matmul(g[:, 3, :], ones_s[:], bhhn[:],
                             start=True, stop=False)
            # recurrent parts: z first (feeds the Pool branch + u), then r
            # (heads the tanh chain), then n.
            for k in range(NK):
                nc.tensor.matmul(g[:, 1, :], hT[:, k, :], whh[:, k, G:2 * G],
                                 start=False, stop=(k == NK - 1))
            for k in range(NK):
                nc.tensor.matmul(g[:, 0, :], hT[:, k, :], whh[:, k, 0:G],
                                 start=False, stop=(k == NK - 1))
            for k in range(NK):
                nc.tensor.matmul(g[:, 3, :], hT[:, k, :],
                                 whh[:, k, 2 * G:3 * G],
                                 start=False, stop=(k == NK - 1))

            # z-route (starts as soon as z mms close)
            nc.scalar.activation(tz[:], g[:, 1, :], AF.Tanh,
                                 bias=zero_b[:], scale=0.5)
            nc.vector.tensor_scalar(u[:], tz[:], -0.5, 0.5, ALU.mult, ALU.add)
            nc.gpsimd.tensor_mul(w1[:], tz[:], h2[:])
            nc.gpsimd.tensor_add(sh[:], w1[:], h2[:])
            # r/n-route: n = tanh(inn + 0.5hn + tr*0.5hn)
            nc.scalar.activation(tr[:], g[:, 0, :], AF.Tanh,
                                 bias=zero_b[:], scale=0.5)
            nc.vector.tensor_mul(m1[:], tr[:], g[:, 3, :])
            nc.vector.tensor_add(q[:], m1[:], g[:, 3, :])
            nc.vector.tensor_add(arg[:], q[:], g[:, 2, :])
            nc.scalar.activation(nn[:], arg[:], AF.Tanh, bias=zero_b[:])
            nc.vector.tensor_mul(nu[:], nn[:], u[:])
            nc.vector.tensor_add(h_sb[:], nu[:], sh[:])

            # h' -> hT: 4 PE transposes + one wide ACT copy (f32 -> f32r)
            for c in range(NK):
                nc.tensor.transpose(tp[:, c, :], h_sb[:, c * 128:(c + 1) * 128],
                                    ident[:])
            nc.scalar.activation(hT[:], tp[:], AF.Copy)
            # refresh h2 = 0.5*h' (off the critical path; Pool)
            nc.gpsimd.tensor_scalar_mul(h2[:], h_sb[:], 0.5)

        # ---------------- encoder ----------------
        for t in range(T):
            x_t = xpool.tile([I + 1, B], f32r)
            dma.dma_start(x_t[:], x_d[t])
            gru_step(x_t[:])

        # ---------------- decoder ----------------
        # y^T [O, B] computed directly: stat = wo chunk [128, O],
        # moving = hT chunk [128, B]; bias added in the ACT copies.
        for t in range(L):
            yt = ypsum.tile([O, B], f32)
            for k in range(NK):
                nc.tensor.matmul(yt[:], wo[:, k, :], hT[:, k, :],
                                 start=(k == 0), stop=(k == NK - 1))
            nc.scalar.activation(outT[:, t * B:(t + 1) * B], yt[:],
                                 AF.Identity, bias=boT[:])
            if t < L - 1:
                nc.scalar.activation(yaug[0:O, :], yt[:], AF.Identity,
                                     bias=boT[:])
                gru_step(yaug[:])

        dma.dma_start(out_d[:], outT[:])

    nc.finalize()
    return nc


def _prep_in_maps(input_, Wih, Whh, bih, bhh, Wo, bo):
    f32 = np.float32
    input_ = np.asarray(input_, f32)
    Wih = np.asarray(Wih, f32)
    Whh = np.asarray(Whh, f32)
    bih = np.asarray(bih, f32)
    bhh = np.asarray(bhh, f32)
    Wo = np.asarray(Wo, f32)
    bo = np.asarray(bo, f32)

    whhT = np.ascontiguousarray(Whh.T)                    # [H, 3H]
    whhT[:, 2 * H:] *= 0.5                                # pre-scale n gate
    wih_aug = np.empty((I + 1, 3 * H), f32)
    wih_aug[:I] = Wih.T                                   # [I, 3H]
    bias_row = bih.copy()
    bias_row[: 2 * H] += bhh[: 2 * H]                     # r,z get bih+bhh
    wih_aug[I] = bias_row                                 # n gets bih only
    bhhn = np.ascontiguousarray(0.5 * bhh[2 * H:].reshape(1, H))
    woT = np.ascontiguousarray(Wo.T)                      # [H, O]
    bo_c = np.ascontiguousarray(bo.reshape(O, 1))

    shared = {"whh": whhT, "wih": wih_aug, "bhhn": bhhn, "wo": woT, "bo": bo_c}
    in_maps = []
    for c in range(N_CORES):
        xc = input_[c * B:(c + 1) * B]                    # [B, T, I]
        xaug = np.empty((T, I + 1, B), f32)
        xaug[:, :I, :] = xc.transpose(1, 2, 0)
        xaug[:, I, :] = 1.0
        in_maps.append({"x": np.ascontiguousarray(xaug), **shared})
    return in_maps


def kernel(**inputs):
    global LAST_RESULTS
    from concourse.bass_utils import run_bass_kernel_spmd

    pl = int(np.asarray(inputs["predict_length"]))
    assert pl == L, f"kernel compiled for predict_length={L}, got {pl}"

    in_maps = _prep_in_maps(
        inputs["input"], inputs["Wih"], inputs["Whh"],
        inputs["bih"], inputs["bhh"], inputs["Wo"], inputs["bo"],
    )

    if "nc" not in _CACHE:
        _CACHE["nc"] = _build_nc()
    nc = _CACHE["nc"]

    res = run_bass_kernel_spmd(nc, in_maps, core_ids=list(range(N_CORES)))
    LAST_RESULTS = res

    out = np.empty((N_CORES * B, L, O), np.float32)
    for c in range(N_CORES):
        out[c * B:(c + 1) * B] = (
            res.results[c]["out"].reshape(O, L, B).transpose(2, 1, 0)
        )
    return out


# revision 35
# speedup vs baseline: 1.2327x; 1.0158x over previous
"""GRU encoder + autoregressive decoder (seq2seq RNN) on 8 TRN2 cores.

Strategy: data-parallel over batch (512 -> 8 x 64), weights replicated.
Per core, the sequential recurrence runs locally:
  - State kept as p = 0.5*h, BOTH batch-major [64, 512] (elementwise) and
    transposed hT [128, 4, 64] f32r (matmul stationary). The 2x is absorbed
    host-side: Whh_{r,z} and Wo are pre-scaled by 2, Whh_n by 1 (its 0.5
    pre-scale cancels), bhh_n by 0.5.
  - Matmuls: out[batch, gate_cols] = hT.T @ W with moving weights in
    float32r (FP22). Biases ride an appended ones-row on the x/y stationary
    plus one [1,512] bhh_n row.
  - sigmoid via tanh: r = 0.5+0.5*tr, z = 0.5+0.5*tz with t* = tanh(0.5*pre).
    n = tanh(inn + (1+tr)*g3), g3 = 0.5*hn.
    p' = nn*u2 + p*v with u2 = 0.25-0.25*tz, v = 0.5+0.5*tz (DVE only; the
    slow GpSimd engine is never used).
  - Emission order: whh matmuls first, x/bias matmuls of the NEXT encoder
    step emitted before this step's transposes so PE fills the elementwise
    tail.
  - Decoder y^T = wo-chunks^T @ hT directly in [O,B] layout; bias folded
    into the ACT copies (one to the f32 output buffer, one to the f32r
    recurrent input).
"""

import numpy as np

N_CORES = 8
B = 64           # batch per core
T = 128          # encoder steps
I = 64           # input size
H = 512          # hidden size
O = 64           # output size
L = 300          # predict length
NK = H // 128    # K tiles over hidden dim
G = 512          # gate region width (cols per gate, = H)

_CACHE = {}
LAST_RESULTS = None


def _build_nc():
    from contextlib import ExitStack

    import concourse.bass as bass
    import concourse.tile as tile
    from concourse import bacc, mybir
    from concourse.masks import make_identity

    f32 = mybir.dt.float32
    f32r = mybir.dt.float32r
    AF = mybir.ActivationFunctionType
    ALU = mybir.AluOpType

    nc = bacc.Bacc(trn_type="TRN2")

    x_d = nc.dram_tensor("x", [T, I + 1, B], f32r, kind="ExternalInput")
    whh_d = nc.dram_tensor("whh", [H, 3 * H], f32r, kind="ExternalInput")
    wih_d = nc.dram_tensor("wih", [I + 1, 3 * H], f32r, kind="ExternalInput")
    bhhn_d = nc.dram_tensor("bhhn", [1, H], f32r, kind="ExternalInput")
    wo_d = nc.dram_tensor("wo", [H, O], f32r, kind="ExternalInput")
    bo_d = nc.dram_tensor("bo", [O, 1], f32, kind="ExternalInput")
    out_d = nc.dram_tensor("out", [O, L * B], f32, kind="ExternalOutput")

    with tile.TileContext(nc) as tc, ExitStack() as ctx:
        singles = ctx.enter_context(tc.tile_pool(name="singles", bufs=1))
        xpool = ctx.enter_context(tc.tile_pool(name="xpool", bufs=3))
        gpsum = ctx.enter_context(tc.tile_pool(name="gpsum", bufs=1, space="PSUM"))
        tpsum = ctx.enter_context(tc.tile_pool(name="tpsum", bufs=2, space="PSUM"))
        ypsum = ctx.enter_context(tc.tile_pool(name="ypsum", bufs=1, space="PSUM"))

        dma = nc.default_dma_engine

        # --- weights / constants ---
        whh = singles.tile([128, NK, 3 * H], f32r)
        dma.dma_start(whh[:], whh_d[:].rearrange("(k p) j -> p k j", p=128))
        wih = singles.tile([I + 1, 3 * H], f32r)
        dma.dma_start(wih[:], wih_d[:])
        bhhn = singles.tile([1, H], f32r)
        dma.dma_start(bhhn[:], bhhn_d[:])
        wo = singles.tile([128, NK, O], f32r)
        dma.dma_start(wo[:], wo_d[:].rearrange("(k p) o -> p k o", p=128))
        boT = singles.tile([O, 1], f32)
        dma.dma_start(boT[:], bo_d[:])

        ones_f = singles.tile([1, B], f32)
        nc.vector.memset(ones_f[:], 1.0)
        ones_s = singles.tile([1, B], f32r)
        nc.scalar.activation(ones_s[:], ones_f[:], AF.Copy)
        zero_b = singles.tile([B, 1], f32)
        nc.vector.memset(zero_b[:], 0.0)
        ident = singles.tile([B, B], f32)
        make_identity(nc, ident[:])

        # --- state: p = 0.5*h ---
        p_sb = singles.tile([B, H], f32)
        nc.vector.memset(p_sb[:], 0.0)
        hz = singles.tile([128, NK, B], f32)
        nc.vector.memset(hz[:], 0.0)
        hT = singles.tile([128, NK, B], f32r)
        nc.scalar.activation(hT[:], hz[:], AF.Copy)
        yz = singles.tile([O + 1, B], f32)
        nc.vector.memset(yz[:], 0.0)
        nc.vector.memset(yz[O:O + 1, :], 1.0)
        yaug = singles.tile([O + 1, B], f32r)
        nc.scalar.activation(yaug[:], yz[:], AF.Copy)
        outT = singles.tile([O, L * B], f32)

        # --- elementwise temporaries (persistent, reused each step) ---
        tz = singles.tile([B, H], f32)
        u2 = singles.tile([B, H], f32)
        v = singles.tile([B, H], f32)
        w = singles.tile([B, H], f32)
        tr = singles.tile([B, H], f32)
        trp = singles.tile([B, H], f32)
        a1 = singles.tile([B, H], f32)
        arg = singles.tile([B, H], f32)
        nn = singles.tile([B, H], f32)
        nu = singles.tile([B, H], f32)

        def new_g():
            # single call site -> single 4-bank PSUM region
            return gpsum.tile([B, 4, G], f32, name="g")

        def gate_x_mms(x_stat, g):
            """x-dependent + bias matmuls (no h dependency). start=True."""
            nc.tensor.matmul(g[:, 2, :], x_stat, wih[:, 2 * G:3 * G],
                             start=True, stop=True)
            nc.tensor.matmul(g[:, 3, :], ones_s[:], bhhn[:],
                             start=True, stop=False)
            nc.tensor.matmul(g[:, 1, :], x_stat, wih[:, G:2 * G],
                             start=True, stop=False)
            nc.tensor.matmul(g[:, 0, :], x_stat, wih[:, 0:G],
                             start=True, stop=False)

        def gate_h_mms(g):
            """Recurrent matmuls; z first (feeds u2/v/w), then r, then n."""
            for k in range(NK):
                nc.tensor.matmul(g[:, 1, :], hT[:, k, :], whh[:, k, G:2 * G],
                                 start=False, stop=(k == NK - 1))
            for k in range(NK):
                nc.tensor.matmul(g[:, 0, :], hT[:, k, :], whh[:, k, 0:G],
                                 start=False, stop=(k == NK - 1))
            for k in range(NK):
                nc.tensor.matmul(g[:, 3, :], hT[:, k, :],
                                 whh[:, k, 2 * G:3 * G],
                                 start=False, stop=(k == NK - 1))

        def el_trans(g, hoist=None):
            """Elementwise chain + h'->hT. hoist() (optional) emits the next
            step's x matmuls between the gate mms and the transposes so PE
            works while the chain drains."""
            nc.scalar.activation(tz[:], g[:, 1, :], AF.Tanh,
                                 bias=zero_b[:], scale=0.5)
            nc.scalar.activation(tr[:], g[:, 0, :], AF.Tanh,
                                 bias=zero_b[:], scale=0.5)
            nc.vector.tensor_scalar(u2[:], tz[:], -0.25, 0.25,
                                    ALU.mult, ALU.add)
            nc.vector.tensor_scalar(v[:], tz[:], 0.5, 0.5,
                                    ALU.mult, ALU.add)
            nc.vector.tensor_mul(w[:], p_sb[:], v[:])
            nc.vector.tensor_scalar_add(trp[:], tr[:], 1.0)
            nc.vector.tensor_mul(a1[:], trp[:], g[:, 3, :])
            nc.vector.tensor_add(arg[:], a1[:], g[:, 2, :])
            nc.scalar.activation(nn[:], arg[:], AF.Tanh, bias=zero_b[:])
            nc.vector.tensor_mul(nu[:], nn[:], u2[:])
            if hoist is not None:
                hoist()
            nc.vector.tensor_add(p_sb[:], nu[:], w[:])
            tp = tpsum.tile([128, NK, B], f32)
            for c in range(NK):
                nc.tensor.transpose(tp[:, c, :], p_sb[:, c * 128:(c + 1) * 128],
                                    ident[:])
            nc.scalar.activation(hT[:], tp[:], AF.Copy)

        # ---------------- encoder ----------------
        x_t = xpool.tile([I + 1, B], f32r)
        dma.dma_start(x_t[:], x_d[0])
        g = new_g()
        gate_x_mms(x_t[:], g)
        for t in range(T):
            gate_h_mms(g)
            if t + 1 < T:
                x_n = xpool.tile([I + 1, B], f32r)
                dma.dma_start(x_n[:], x_d[t + 1])
                g_n = new_g()

                def hoist(x_n=x_n, g_n=g_n):
                    gate_x_mms(x_n[:], g_n)
            else:
                g_n = None
                hoist = None
            el_trans(g, hoist)
            g = g_n

        # ---------------- decoder ----------------
        for t in range(L):
            yt = ypsum.tile([O, B], f32)
            for k in range(NK):
                nc.tensor.matmul(yt[:], wo[:, k, :], hT[:, k, :],
                                 start=(k == 0), stop=(k == NK - 1))
            nc.scalar.activation(outT[:, t * B:(t + 1) * B], yt[:],
                                 AF.Identity, bias=boT[:])
            if t < L - 1:
                nc.scalar.activation(yaug[0:O, :], yt[:], AF.Identity,
                                     bias=boT[:])
                g = new_g()
                gate_x_mms(yaug[:], g)
                gate_h_mms(g)
                el_trans(g)

        dma.dma_start(out_d[:], outT[:])

    nc.finalize()
    return nc


def _prep_in_maps(input_, Wih, Whh, bih, bhh, Wo, bo):
    f32 = np.float32
    input_ = np.asarray(input_, f32)
    Wih = np.asarray(Wih, f32)
    Whh = np.asarray(Whh, f32)
    bih = np.asarray(bih, f32)
    bhh = np.asarray(bhh, f32)
    Wo = np.asarray(Wo, f32)
    bo = np.asarray(bo, f32)

    whhT = np.ascontiguousarray(Whh.T)                    # [H, 3H]
    whhT[:, :2 * H] *= 2.0                                # p = 0.5h absorbs 2x
    wih_aug = np.empty((I + 1, 3 * H), f32)
    wih_aug[:I] = Wih.T                                   # [I, 3H]
    bias_row = bih.copy()
    bias_row[: 2 * H] += bhh[: 2 * H]                     # r,z get bih+bhh
    wih_aug[I] = bias_row                                 # n gets bih only
    bhhn = np.ascontiguousarray(0.5 * bhh[2 * H:].reshape(1, H))
    woT = np.ascontiguousarray(2.0 * Wo.T)                # [H, O], 2x for p
    bo_c = np.ascontiguousarray(bo.reshape(O, 1))

    shared = {"whh": whhT, "wih": wih_aug, "bhhn": bhhn, "wo": woT, "bo": bo_c}
    in_maps = []
    for c in range(N_CORES):
        xc = input_[c * B:(c + 1) * B]                    # [B, T, I]
        xaug = np.empty((T, I + 1, B), f32)
        xaug[:, :I, :] = xc.transpose(1, 2, 0)
        xaug[:, I, :] = 1.0
        in_maps.append({"x": np.ascontiguousarray(xaug), **shared})
    return in_maps


def kernel(**inputs):
    global LAST_RESULTS
    from concourse.bass_utils import run_bass_kernel_spmd

    pl = int(np.asarray(inputs["predict_length"]))
    assert pl == L, f"kernel compiled for predict_length={L}, got {pl}"

    in_maps = _prep_in_maps(
        inputs["input"], inputs["Wih"], inputs["Whh"],
        inputs["bih"], inputs["bhh"], inputs["Wo"], inputs["bo"],
    )

    if "nc" not in _CACHE:
        _CACHE["nc"] = _build_nc()
    nc = _CACHE["nc"]

    res = run_bass_kernel_spmd(nc, in_maps, core_ids=list(range(N_CORES)))
    LAST_RESULTS = res

    out = np.empty((N_CORES * B, L, O), np.float32)
    for c in range(N_CORES):
        out[c * B:(c + 1) * B] = (
            res.results[c]["out"].reshape(O, L, B).transpose(2, 1, 0)
        )
    return out
